# revision 16
# baseline (speedup 1.0000x reference)
"""DCTHFClip kernel for 8 Trainium2 NeuronCores — DCT butterfly edition.

Math: the reference computes
    x_dct   = C @ x          (DCT-II along S, per (batch, feature) column)
    m       = |mean_{b,d} x_dct|          (shape (S,))
    thr     = quantile(m, 0.7); last_index = last k with m[k] > thr
    trunc   = x_dct[:, :L, :]                           (fp32 output)
    recon   = Cl^T @ trunc  with Cl = dct_matrix(L)     (fp16 output)

Key reductions:
  1. m is LINEAR in x:  m = |C @ mean_{b,d}(x)| — resolved on host, so L
     is static before any device work (mirrors the host .item() sync).
  2. DCT-II butterfly (row parity of C): with f = x[:S/2] and
     b_rev = x[S-1 : S/2-1 : -1] (HOST-side reversal — free),
        u = f + b_rev, v = f - b_rev              (DVE adds)
        trunc[2j]   = (Ce @ u)[j] = yE[j]          (287x288 matmul)
        trunc[2j+1] = (Co @ v)[j] = yO[j]          (287x288 matmul)
     and the IDCT butterfly (row parity of Cl in position):
        a = Ae @ yE, b = Ao @ yO                   (287x287 matmuls)
        recon[p]     = a[p] + b[p]                 p < L/2
        recon[L-1-p] = a[p] - b[p]                 (HOST flips the back)
     Total PE work is HALF the fused single-matmul formulation.
  3. trunc leaves the device as fp16 (the HOST upcasts to fp32): the
     fp16 round adds ~5e-4 relative error (inside tolerance), halves
     the trunc HBM traffic, and lets ONE scalar-engine cast per PSUM
     tile serve both the trunc DMA and the stage-B matmul input.
  4. Interleaved trunc rows are written with stride-2 DRAM row APs
     (positive steps only — the BIR verifier rejects negative ones).

Scheduling notes (HAM: the PE clock-gate drops to 1.2 GHz after any
~3.4us idle window, so the PE must stay continuously fed):
  - PSUM tiles are [128, 1024] fp32 (2 banks), one rotating pool of 4
    shared by both stages; each tile is one 3-matmul accumulation chain
    with moving dim 1024, evacuated by a single engine op.
  - Stage B of batch b is emitted after stage A of batch b+1 so the PE
    has work while the casts of batch b drain.
  - The 32/31-row contraction remainders ride disjoint PE row groups
    (partitions 0:32 and 32:64) and execute concurrently.
  - fp32->fp16 casts run on the scalar engine (the DVE cast path
    intermittently emits top-nibble-quantized values under load);
    fp16 add/sub combines run on the vector engine at 16-bit rate.
"""

import os
import sys

import numpy as np

_B, _S, _D = 64, 576, 1024
_NCORES = 8
_P = 128          # SBUF partitions

_CACHE = {}
LAST_RESULTS = None  # stashed BassKernelResults for test.py profiling


def _ensure_paths():
    for p in ("/root/.axon_site", "/root/.axon_site/_ro/trn_rl_repo",
              "/root/.axon_site/_ro/pypackages", "/opt/trn_rl_repo", "/opt/pypackages"):
        if os.path.isdir(p) and p not in sys.path:
            sys.path.append(p)


def _dct_matrix64(n):
    k = np.arange(n)[:, None].astype(np.float64)
    i = np.arange(n)[None, :].astype(np.float64)
    C = np.cos(np.pi / n * (i + 0.5) * k)
    scale = np.where(k == 0, np.sqrt(1.0 / n), np.sqrt(2.0 / n))
    return C * scale  # (n_freq, n_pos)


def _resolve_L(x):
    """Host-side: trunc length via linearity of the batch/feature mean."""
    S = x.shape[1]
    xbar = x.mean(axis=(0, 2), dtype=np.float64)  # (S,)
    C = _dct_matrix64(S)
    m = np.abs(C @ xbar)
    thr = np.quantile(m, 0.7)
    idx = np.nonzero(m > thr)[0]
    last_index = int(idx[-1]) if idx.size > 0 else -1
    # mirror python slice semantics of x_dct[:, :last_index, :]
    return len(range(S)[:last_index])


def _build_weights(S, L):
    """fp16 lhsT weight arrays for the butterfly formulation."""
    H = S // 2            # 288
    F = L // 2            # 287
    C = _dct_matrix64(S)
    Cl = _dct_matrix64(L)
    # stage A: yE = Ce @ u, yO = Co @ v;  Ce[j,i] = C[2j,i], Co[j,i] = C[2j+1,i]
    CeT = C[0:2 * F:2, 0:H].T       # (H, F) lhsT
    CoT = C[1:2 * F:2, 0:H].T       # (H, F)
    # stage B: a = Ae @ yE, b = Ao @ yO;  Ae[p,j] = Cl[2j,p], Ao[p,j] = Cl[2j+1,p]
    AeT = Cl[0::2, 0:F]             # (F, F) lhsT (rows j, cols p)
    AoT = Cl[1::2, 0:F]             # (F, F)
    f16 = np.float16
    kfA = H // _P                   # 2 full k-tiles
    krA = H - kfA * _P              # 32
    kfB = F // _P                   # 2
    krB = F - kfB * _P              # 31
    waE = np.ascontiguousarray(CeT[:kfA * _P], f16)
    waO = np.ascontiguousarray(CoT[:kfA * _P], f16)
    # remainders at partition offsets 0 and 32 (zero-pad odd-sized ones)
    wrA = np.zeros((64, F), f16)
    wrA[0:krA] = CeT[kfA * _P:]
    wrA[32:32 + krA] = CoT[kfA * _P:]
    wbE = np.ascontiguousarray(AeT[:kfB * _P], f16)
    wbO = np.ascontiguousarray(AoT[:kfB * _P], f16)
    wrB = np.zeros((64, F), f16)
    wrB[0:krB] = AeT[kfB * _P:]
    wrB[32:32 + krB] = AoT[kfB * _P:]
    return dict(waE=waE, waO=waO, wremA=wrA, wbE=wbE, wbO=wbO, wremB=wrB)


def _build_program(Bc, S, D, L):
    _ensure_paths()
    import concourse.bacc as bacc
    import concourse.mybir as mybir
    import concourse.tile as tile

    f32 = mybir.dt.float32
    f16 = mybir.dt.float16

    P = _P
    H = S // 2                  # 288 contraction (stage A)
    F = L // 2                  # 287 rows per parity / contraction (stage B)
    kfA = H // P                # 2 full k-tiles stage A
    krA = H - kfA * P           # 32 remainder rows
    kfB = F // P                # 2 full k-tiles stage B
    krB = F - kfB * P           # 31 remainder rows
    RT = (F + P - 1) // P       # 3 row tiles (128,128,31)

    nc = bacc.Bacc("TRN2", target_bir_lowering=False, debug=False,
                   num_devices=_NCORES)
    xf_d = nc.dram_tensor("xf", [Bc, H, D], f16, kind="ExternalInput")
    xb_d = nc.dram_tensor("xb", [Bc, H, D], f16, kind="ExternalInput")
    waE_d = nc.dram_tensor("waE", [kfA * P, F], f16, kind="ExternalInput")
    waO_d = nc.dram_tensor("waO", [kfA * P, F], f16, kind="ExternalInput")
    wrA_d = nc.dram_tensor("wremA", [64, F], f16, kind="ExternalInput")
    wbE_d = nc.dram_tensor("wbE", [kfB * P, F], f16, kind="ExternalInput")
    wbO_d = nc.dram_tensor("wbO", [kfB * P, F], f16, kind="ExternalInput")
    wrB_d = nc.dram_tensor("wremB", [64, F], f16, kind="ExternalInput")
    tr_d = nc.dram_tensor("trunc16", [Bc, L, D], f16, kind="ExternalOutput")
    rf_d = nc.dram_tensor("rfront", [Bc, F, D], f16, kind="ExternalOutput")
    rb_d = nc.dram_tensor("rback", [Bc, F, D], f16, kind="ExternalOutput")

    with tile.TileContext(nc) as tc:
        with (
            tc.tile_pool(name="wpool", bufs=1) as wpool,
            tc.tile_pool(name="xpool", bufs=2) as xpool,
            tc.tile_pool(name="uvpool", bufs=2) as uvpool,
            tc.tile_pool(name="ypool", bufs=2) as ypool,
            tc.tile_pool(name="abpool", bufs=2) as abpool,
            tc.tile_pool(name="rpool", bufs=2) as rpool,
            # rotating quad of [128, 1024] fp32 tiles (2 banks each)
            # shared by both stages: depth-4 PE pipeline, 8 banks total
            tc.tile_pool(name="psum", bufs=4, space="PSUM") as psum_pool,
        ):
            # ---- weights (scalar ring; tiny) ------------------------------
            waE_t = wpool.tile([P, kfA, F], f16)
            waO_t = wpool.tile([P, kfA, F], f16)
            wrA_t = wpool.tile([64, F], f16)
            wbE_t = wpool.tile([P, kfB, F], f16)
            wbO_t = wpool.tile([P, kfB, F], f16)
            wrB_t = wpool.tile([64, F], f16)
            nc.scalar.dma_start(
                out=waE_t, in_=waE_d[:, :].rearrange("(kt p) m -> p kt m", p=P))
            nc.scalar.dma_start(
                out=waO_t, in_=waO_d[:, :].rearrange("(kt p) m -> p kt m", p=P))
            nc.scalar.dma_start(out=wrA_t, in_=wrA_d[:, :])
            nc.scalar.dma_start(
                out=wbE_t, in_=wbE_d[:, :].rearrange("(kt p) m -> p kt m", p=P))
            nc.scalar.dma_start(
                out=wbO_t, in_=wbO_d[:, :].rearrange("(kt p) m -> p kt m", p=P))
            nc.scalar.dma_start(out=wrB_t, in_=wrB_d[:, :])

            def stage_a_pre(b):
                # x loads (sync ring), per k-tile so the butterfly starts
                # after the first 512KB instead of the full 1.2MB
                xf_t = xpool.tile([P, kfA, D], f16, tag="xf")
                xb_t = xpool.tile([P, kfA, D], f16, tag="xb")
                # remainder rows duplicated at partition offsets 0 and 32 so
                # the u/v remainder ops see matching in/out partition bases
                xr_t = xpool.tile([64, 2, D], f16, tag="xr")
                nc.sync.dma_start(
                    out=xf_t[:, :, :],
                    in_=xf_d[b, 0:kfA * P, :].rearrange(
                        "(kt p) d -> p kt d", p=P))
                nc.sync.dma_start(
                    out=xb_t[:, :, :],
                    in_=xb_d[b, 0:kfA * P, :].rearrange(
                        "(kt p) d -> p kt d", p=P))
                nc.sync.dma_start(out=xr_t[0:krA, 0, :],
                                  in_=xf_d[b, kfA * P:H, :])
                nc.sync.dma_start(out=xr_t[0:krA, 1, :],
                                  in_=xb_d[b, kfA * P:H, :])
                nc.sync.dma_start(out=xr_t[32:32 + krA, 0, :],
                                  in_=xf_d[b, kfA * P:H, :])
                nc.sync.dma_start(out=xr_t[32:32 + krA, 1, :],
                                  in_=xb_d[b, kfA * P:H, :])

                # butterfly u,v (DVE; fp16 SBUF 2x rate)
                u_t = uvpool.tile([P, kfA, D], f16, tag="u")
                v_t = uvpool.tile([P, kfA, D], f16, tag="v")
                uvr_t = uvpool.tile([64, D], f16, tag="uvr")
                nc.vector.tensor_add(u_t[:, :, :], xf_t[:, :, :],
                                     xb_t[:, :, :])
                nc.vector.tensor_sub(v_t[:, :, :], xf_t[:, :, :],
                                     xb_t[:, :, :])
                nc.vector.tensor_add(uvr_t[0:krA, :], xr_t[0:krA, 0, :],
                                     xr_t[0:krA, 1, :])
                nc.vector.tensor_sub(uvr_t[32:32 + krA, :],
                                     xr_t[32:32 + krA, 0, :],
                                     xr_t[32:32 + krA, 1, :])

                # stage A: one 3-matmul chain per (rt, parity), moving 1024;
                # the 31-row tail packs yE at partitions 0:31 and yO at
                # 32:63 of ONE tile so a single base-0 cast evacuates both
                y16E = ypool.tile([P, kfB, D], f16, tag="y16E")
                y16O = ypool.tile([P, kfB, D], f16, tag="y16O")
                yr16 = ypool.tile([64, D], f16, tag="yr16")
                return u_t, v_t, uvr_t, y16E, y16O, yr16

            def stage_a_rt(rt, ta):
                u_t, v_t, uvr_t, y16E, y16O, yr16 = ta
                if True:
                    r0 = rt * P
                    rows = min(P, F - r0)
                    tail = rt >= kfB
                    pe = psum_pool.tile([P, D], f32, tag="ps", name="pe")
                    po = pe if tail else psum_pool.tile([P, D], f32,
                                                        tag="ps", name="po")
                    oP = 32 if tail else 0
                    for ng in range(2):
                        n0 = ng * 512
                        n1 = n0 + 512
                        for kt in range(kfA):
                            nc.tensor.matmul(pe[0:rows, n0:n1],
                                             waE_t[:, kt, r0:r0 + rows],
                                             u_t[:, kt, n0:n1],
                                             start=(kt == 0), stop=False)
                            nc.tensor.matmul(po[oP:oP + rows, n0:n1],
                                             waO_t[:, kt, r0:r0 + rows],
                                             v_t[:, kt, n0:n1],
                                             start=(kt == 0), stop=False)
                        nc.tensor.matmul(pe[0:rows, n0:n1],
                                         wrA_t[0:krA, r0:r0 + rows],
                                         uvr_t[0:krA, n0:n1],
                                         start=False, stop=True)
                        nc.tensor.matmul(po[oP:oP + rows, n0:n1],
                                         wrA_t[32:32 + krA, r0:r0 + rows],
                                         uvr_t[32:32 + krA, n0:n1],
                                         start=False, stop=True)
                    # single fp16 cast per tile (ACT) = trunc staging AND
                    # stage-B matmul input
                    if not tail:
                        nc.scalar.copy(y16E[0:rows, rt, :], pe[0:rows, :])
                        nc.scalar.copy(y16O[0:rows, rt, :], po[0:rows, :])
                    else:
                        nc.scalar.copy(yr16[0:32 + rows, :],
                                       pe[0:32 + rows, :])

            def stage_a_out(b, ta):
                y16E, y16O, yr16 = ta[3], ta[4], ta[5]
                # trunc out, fp16 (sync ring): row = 2*(kt*128+p) + parity
                nc.sync.dma_start(
                    out=tr_d[b, 0:2 * kfB * P:2, :].rearrange(
                        "(kt p) d -> p kt d", p=P),
                    in_=y16E[:, 0:kfB, :])
                nc.sync.dma_start(
                    out=tr_d[b, 1:2 * kfB * P:2, :].rearrange(
                        "(kt p) d -> p kt d", p=P),
                    in_=y16O[:, 0:kfB, :])
                nc.sync.dma_start(out=tr_d[b, 4 * P:L:2, :],
                                  in_=yr16[0:krB, :])
                nc.sync.dma_start(out=tr_d[b, 4 * P + 1:L:2, :],
                                  in_=yr16[32:32 + krB, :])

            def stage_b_pre():
                a16 = abpool.tile([P, RT, D], f16, tag="a16")
                b16 = abpool.tile([P, RT, D], f16, tag="b16")
                rf_sb = rpool.tile([P, RT, D], f16, tag="rf")
                rb_sb = rpool.tile([P, RT, D], f16, tag="rb")
                return a16, b16, rf_sb, rb_sb

            def stage_b_rt(rt, ta, tb):
                y16E, y16O, yr16 = ta[3], ta[4], ta[5]
                a16, b16, rf_sb, rb_sb = tb
                if True:
                    r0 = rt * P
                    rows = min(P, F - r0)
                    pa = psum_pool.tile([P, D], f32, tag="ps", name="pa")
                    pb = psum_pool.tile([P, D], f32, tag="ps", name="pb")
                    for ng in range(2):
                        n0 = ng * 512
                        n1 = n0 + 512
                        for kt in range(kfB):
                            nc.tensor.matmul(pa[0:rows, n0:n1],
                                             wbE_t[:, kt, r0:r0 + rows],
                                             y16E[:, kt, n0:n1],
                                             start=(kt == 0), stop=False)
                            nc.tensor.matmul(pb[0:rows, n0:n1],
                                             wbO_t[:, kt, r0:r0 + rows],
                                             y16O[:, kt, n0:n1],
                                             start=(kt == 0), stop=False)
                        nc.tensor.matmul(pa[0:rows, n0:n1],
                                         wrB_t[0:krB, r0:r0 + rows],
                                         yr16[0:krB, n0:n1],
                                         start=False, stop=True)
                        nc.tensor.matmul(pb[0:rows, n0:n1],
                                         wrB_t[32:32 + krB, r0:r0 + rows],
                                         yr16[32:32 + krB, n0:n1],
                                         start=False, stop=True)
                    # PSUM evacuation split across both engines so neither
                    # queue gates the tile rotation; combines at fp16 rate
                    nc.scalar.copy(a16[0:rows, rt, :], pa[0:rows, :])
                    nc.vector.tensor_copy(b16[0:rows, rt, :], pb[0:rows, :])
                    nc.vector.tensor_add(rf_sb[0:rows, rt, :],
                                         a16[0:rows, rt, :],
                                         b16[0:rows, rt, :])
                    nc.vector.tensor_sub(rb_sb[0:rows, rt, :],
                                         a16[0:rows, rt, :],
                                         b16[0:rows, rt, :])

            def stage_b_out(b, tb):
                rf_sb, rb_sb = tb[2], tb[3]
                # recon out (gpsimd SWDGE — the known-safe fp16 path)
                nc.gpsimd.dma_start(
                    out=rf_d[b, 0:kfB * P, :].rearrange(
                        "(kt p) d -> p kt d", p=P),
                    in_=rf_sb[:, 0:kfB, :])
                nc.gpsimd.dma_start(out=rf_d[b, kfB * P:F, :],
                                    in_=rf_sb[0:krB, kfB, :])
                nc.gpsimd.dma_start(
                    out=rb_d[b, 0:kfB * P, :].rearrange(
                        "(kt p) d -> p kt d", p=P),
                    in_=rb_sb[:, 0:kfB, :])
                nc.gpsimd.dma_start(out=rb_d[b, kfB * P:F, :],
                                    in_=rb_sb[0:krB, kfB, :])

            # stage B of batch b-1 runs after stage A of batch b so the
            # PE has work while the fp16 casts of batch b drain
            prev = None
            for b in range(Bc):
                ta = stage_a_pre(b)
                for rt in range(RT):
                    stage_a_rt(rt, ta)
                stage_a_out(b, ta)
                if prev is not None:
                    tb = stage_b_pre()
                    for rt in range(RT):
                        stage_b_rt(rt, prev, tb)
                    stage_b_out(b - 1, tb)
                prev = ta
            tb = stage_b_pre()
            for rt in range(RT):
                stage_b_rt(rt, prev, tb)
            stage_b_out(Bc - 1, tb)

    nc.compile()
    return nc


def _numpy_fallback(x):
    """Reference math on host — only for unexpected shapes/degenerate L."""
    B, S, D = x.shape
    C = _dct_matrix64(S).astype(np.float32)
    x_dct = np.tensordot(C, x, axes=([1], [1])).transpose(1, 0, 2)  # (B,S,D)
    m = np.abs(x_dct.mean(axis=0).mean(axis=1))
    thr = np.quantile(m, 0.7)
    idx = np.nonzero(m > thr)[0]
    last_index = int(idx[-1]) if idx.size > 0 else -1
    trunc = x_dct[:, :last_index, :]
    L = trunc.shape[1]
    Cl = _dct_matrix64(L).astype(np.float32)
    recon = np.tensordot(Cl.T, trunc, axes=([1], [1])).transpose(1, 0, 2)
    return recon.astype(np.float16), np.ascontiguousarray(trunc)


def kernel(x, _trace=False):
    global LAST_RESULTS
    x = np.ascontiguousarray(np.asarray(x), dtype=np.float32)
    if x.shape != (_B, _S, _D):
        return _numpy_fallback(x)

    L = _resolve_L(x)
    # butterfly path needs even L with L/2 in (2*P, 3*P)
    if L % 2 or not (2 * _P < L // 2 < 3 * _P) or L >= _S:
        return _numpy_fallback(x)

    Bc = _B // _NCORES
    key = (Bc, _S, _D, L)
    if key not in _CACHE:
        _CACHE[key] = _build_program(Bc, _S, _D, L)
    nc = _CACHE[key]

    _ensure_paths()
    if not _trace:
        os.environ["BASS_NEVER_TRACE"] = "1"
    from concourse.bass_utils import run_bass_kernel_spmd

    W = _build_weights(_S, L)
    H, F = _S // 2, L // 2
    x16 = x.astype(np.float16)
    xf = np.ascontiguousarray(x16[:, :H, :])
    xb = np.ascontiguousarray(x16[:, _S - 1:H - 1:-1, :])
    in_maps = []
    for i in range(_NCORES):
        m = {"xf": xf[i * Bc:(i + 1) * Bc], "xb": xb[i * Bc:(i + 1) * Bc]}
        m.update(W)
        in_maps.append(m)
    res = run_bass_kernel_spmd(nc, in_maps, list(range(_NCORES)), trace=_trace)
    LAST_RESULTS = res

    trunc = np.empty((_B, L, _D), dtype=np.float32)
    recon = np.empty((_B, L, _D), dtype=np.float16)
    for i in range(_NCORES):
        sl = slice(i * Bc, (i + 1) * Bc)
        trunc[sl] = res.results[i]["trunc16"]
        recon[sl, :F] = res.results[i]["rfront"]
        recon[sl, F:] = res.results[i]["rback"][:, ::-1]
    return recon, trunc


# revision 17
# speedup vs baseline: 1.0603x; 1.0603x over previous
"""DCTHFClip kernel for 8 Trainium2 NeuronCores — DCT butterfly edition.

Math: the reference computes
    x_dct   = C @ x          (DCT-II along S, per (batch, feature) column)
    m       = |mean_{b,d} x_dct|          (shape (S,))
    thr     = quantile(m, 0.7); last_index = last k with m[k] > thr
    trunc   = x_dct[:, :L, :]                           (fp32 output)
    recon   = Cl^T @ trunc  with Cl = dct_matrix(L)     (fp16 output)

Key reductions:
  1. m is LINEAR in x:  m = |C @ mean_{b,d}(x)| — resolved on host, so L
     is static before any device work (mirrors the host .item() sync).
  2. DCT-II butterfly (row parity of C): with f = x[:S/2] and
     b_rev = x[S-1 : S/2-1 : -1] (HOST-side reversal — free),
        u = f + b_rev, v = f - b_rev              (DVE adds)
        trunc[2j]   = (Ce @ u)[j] = yE[j]          (287x288 matmul)
        trunc[2j+1] = (Co @ v)[j] = yO[j]          (287x288 matmul)
     and the IDCT butterfly (row parity of Cl in position):
        a = Ae @ yE, b = Ao @ yO                   (287x287 matmuls)
        recon[p]     = a[p] + b[p]                 p < L/2
        recon[L-1-p] = a[p] - b[p]                 (HOST flips the back)
     Total PE work is HALF the fused single-matmul formulation.
  3. trunc leaves the device as fp16 (the HOST upcasts to fp32): the
     fp16 round adds ~5e-4 relative error (inside tolerance), halves
     the trunc HBM traffic, and lets ONE scalar-engine cast per PSUM
     tile serve both the trunc DMA and the stage-B matmul input.
  4. Interleaved trunc rows are written with stride-2 DRAM row APs
     (positive steps only — the BIR verifier rejects negative ones).

Scheduling notes (HAM: the PE clock-gate drops to 1.2 GHz after any
~3.4us idle window, so the PE must stay continuously fed):
  - PSUM tiles are [128, 1024] fp32 (2 banks), one rotating pool of 4
    shared by both stages; each tile is one 3-matmul accumulation chain
    with moving dim 1024, evacuated by a single engine op.
  - Stage B of batch b is emitted after stage A of batch b+1 so the PE
    has work while the casts of batch b drain.
  - The 32/31-row contraction remainders ride disjoint PE row groups
    (partitions 0:32 and 32:64) and execute concurrently.
  - fp32->fp16 casts run on the scalar engine (the DVE cast path
    intermittently emits top-nibble-quantized values under load);
    fp16 add/sub combines run on the vector engine at 16-bit rate.
"""

import os
import sys

import numpy as np

_B, _S, _D = 64, 576, 1024
_NCORES = 8
_P = 128          # SBUF partitions

_CACHE = {}
LAST_RESULTS = None  # stashed BassKernelResults for test.py profiling


def _ensure_paths():
    for p in ("/root/.axon_site", "/root/.axon_site/_ro/trn_rl_repo",
              "/root/.axon_site/_ro/pypackages", "/opt/trn_rl_repo", "/opt/pypackages"):
        if os.path.isdir(p) and p not in sys.path:
            sys.path.append(p)


def _dct_matrix64(n):
    k = np.arange(n)[:, None].astype(np.float64)
    i = np.arange(n)[None, :].astype(np.float64)
    C = np.cos(np.pi / n * (i + 0.5) * k)
    scale = np.where(k == 0, np.sqrt(1.0 / n), np.sqrt(2.0 / n))
    return C * scale  # (n_freq, n_pos)


def _resolve_L(x):
    """Host-side: trunc length via linearity of the batch/feature mean."""
    S = x.shape[1]
    xbar = x.mean(axis=(0, 2), dtype=np.float64)  # (S,)
    C = _dct_matrix64(S)
    m = np.abs(C @ xbar)
    thr = np.quantile(m, 0.7)
    idx = np.nonzero(m > thr)[0]
    last_index = int(idx[-1]) if idx.size > 0 else -1
    # mirror python slice semantics of x_dct[:, :last_index, :]
    return len(range(S)[:last_index])


def _build_weights(S, L):
    """fp16 lhsT weight arrays for the butterfly formulation."""
    H = S // 2            # 288
    F = L // 2            # 287
    C = _dct_matrix64(S)
    Cl = _dct_matrix64(L)
    # stage A: yE = Ce @ u, yO = Co @ v;  Ce[j,i] = C[2j,i], Co[j,i] = C[2j+1,i]
    CeT = C[0:2 * F:2, 0:H].T       # (H, F) lhsT
    CoT = C[1:2 * F:2, 0:H].T       # (H, F)
    # stage B: a = Ae @ yE, b = Ao @ yO;  Ae[p,j] = Cl[2j,p], Ao[p,j] = Cl[2j+1,p]
    AeT = Cl[0::2, 0:F]             # (F, F) lhsT (rows j, cols p)
    AoT = Cl[1::2, 0:F]             # (F, F)
    f16 = np.float16
    kfA = H // _P                   # 2 full k-tiles
    krA = H - kfA * _P              # 32
    kfB = F // _P                   # 2
    krB = F - kfB * _P              # 31
    waE = np.ascontiguousarray(CeT[:kfA * _P], f16)
    waO = np.ascontiguousarray(CoT[:kfA * _P], f16)
    # remainders at partition offsets 0 and 32 (zero-pad odd-sized ones)
    wrA = np.zeros((64, F), f16)
    wrA[0:krA] = CeT[kfA * _P:]
    wrA[32:32 + krA] = CoT[kfA * _P:]
    wbE = np.ascontiguousarray(AeT[:kfB * _P], f16)
    wbO = np.ascontiguousarray(AoT[:kfB * _P], f16)
    wrB = np.zeros((64, F), f16)
    wrB[0:krB] = AeT[kfB * _P:]
    wrB[32:32 + krB] = AoT[kfB * _P:]
    return dict(waE=waE, waO=waO, wremA=wrA, wbE=wbE, wbO=wbO, wremB=wrB)


def _build_program(Bc, S, D, L):
    _ensure_paths()
    import concourse.bacc as bacc
    import concourse.mybir as mybir
    import concourse.tile as tile

    f32 = mybir.dt.float32
    f16 = mybir.dt.float16

    P = _P
    H = S // 2                  # 288 contraction (stage A)
    F = L // 2                  # 287 rows per parity / contraction (stage B)
    kfA = H // P                # 2 full k-tiles stage A
    krA = H - kfA * P           # 32 remainder rows
    kfB = F // P                # 2 full k-tiles stage B
    krB = F - kfB * P           # 31 remainder rows
    RT = (F + P - 1) // P       # 3 row tiles (128,128,31)

    nc = bacc.Bacc("TRN2", target_bir_lowering=False, debug=False,
                   num_devices=_NCORES)
    xf_d = nc.dram_tensor("xf", [Bc, H, D], f16, kind="ExternalInput")
    xb_d = nc.dram_tensor("xb", [Bc, H, D], f16, kind="ExternalInput")
    waE_d = nc.dram_tensor("waE", [kfA * P, F], f16, kind="ExternalInput")
    waO_d = nc.dram_tensor("waO", [kfA * P, F], f16, kind="ExternalInput")
    wrA_d = nc.dram_tensor("wremA", [64, F], f16, kind="ExternalInput")
    wbE_d = nc.dram_tensor("wbE", [kfB * P, F], f16, kind="ExternalInput")
    wbO_d = nc.dram_tensor("wbO", [kfB * P, F], f16, kind="ExternalInput")
    wrB_d = nc.dram_tensor("wremB", [64, F], f16, kind="ExternalInput")
    tr_d = nc.dram_tensor("trunc16", [Bc, L, D], f16, kind="ExternalOutput")
    rf_d = nc.dram_tensor("rfront", [Bc, F, D], f16, kind="ExternalOutput")
    rb_d = nc.dram_tensor("rback", [Bc, F, D], f16, kind="ExternalOutput")

    with tile.TileContext(nc) as tc:
        with (
            tc.tile_pool(name="wpool", bufs=1) as wpool,
            tc.tile_pool(name="xpool", bufs=2) as xpool,
            tc.tile_pool(name="uvpool", bufs=2) as uvpool,
            tc.tile_pool(name="ypool", bufs=2) as ypool,
            tc.tile_pool(name="abpool", bufs=2) as abpool,
            tc.tile_pool(name="rpool", bufs=2) as rpool,
            # rotating quad of [128, 1024] fp32 tiles (2 banks each)
            # shared by both stages: depth-4 PE pipeline, 8 banks total
            tc.tile_pool(name="psum", bufs=4, space="PSUM") as psum_pool,
        ):
            # ---- weights (scalar ring; tiny) ------------------------------
            waE_t = wpool.tile([P, kfA, F], f16)
            waO_t = wpool.tile([P, kfA, F], f16)
            wrA_t = wpool.tile([64, F], f16)
            wbE_t = wpool.tile([P, kfB, F], f16)
            wbO_t = wpool.tile([P, kfB, F], f16)
            wrB_t = wpool.tile([64, F], f16)
            nc.scalar.dma_start(
                out=waE_t, in_=waE_d[:, :].rearrange("(kt p) m -> p kt m", p=P))
            nc.scalar.dma_start(
                out=waO_t, in_=waO_d[:, :].rearrange("(kt p) m -> p kt m", p=P))
            nc.scalar.dma_start(out=wrA_t, in_=wrA_d[:, :])
            nc.scalar.dma_start(
                out=wbE_t, in_=wbE_d[:, :].rearrange("(kt p) m -> p kt m", p=P))
            nc.scalar.dma_start(
                out=wbO_t, in_=wbO_d[:, :].rearrange("(kt p) m -> p kt m", p=P))
            nc.scalar.dma_start(out=wrB_t, in_=wrB_d[:, :])

            def stage_a_pre(b):
                # x loads (sync ring), per k-tile so the butterfly starts
                # after the first 512KB instead of the full 1.2MB
                xf_t = xpool.tile([P, kfA, D], f16, tag="xf")
                xb_t = xpool.tile([P, kfA, D], f16, tag="xb")
                # remainder rows duplicated at partition offsets 0 and 32 so
                # the u/v remainder ops see matching in/out partition bases
                xr_t = xpool.tile([64, 2, D], f16, tag="xr")
                nc.sync.dma_start(
                    out=xf_t[:, :, :],
                    in_=xf_d[b, 0:kfA * P, :].rearrange(
                        "(kt p) d -> p kt d", p=P))
                nc.sync.dma_start(
                    out=xb_t[:, :, :],
                    in_=xb_d[b, 0:kfA * P, :].rearrange(
                        "(kt p) d -> p kt d", p=P))
                nc.sync.dma_start(out=xr_t[0:krA, 0, :],
                                  in_=xf_d[b, kfA * P:H, :])
                nc.sync.dma_start(out=xr_t[0:krA, 1, :],
                                  in_=xb_d[b, kfA * P:H, :])
                nc.sync.dma_start(out=xr_t[32:32 + krA, 0, :],
                                  in_=xf_d[b, kfA * P:H, :])
                nc.sync.dma_start(out=xr_t[32:32 + krA, 1, :],
                                  in_=xb_d[b, kfA * P:H, :])

                # butterfly u,v (DVE; fp16 SBUF 2x rate)
                u_t = uvpool.tile([P, kfA, D], f16, tag="u")
                v_t = uvpool.tile([P, kfA, D], f16, tag="v")
                uvr_t = uvpool.tile([64, D], f16, tag="uvr")
                nc.vector.tensor_add(u_t[:, :, :], xf_t[:, :, :],
                                     xb_t[:, :, :])
                nc.vector.tensor_sub(v_t[:, :, :], xf_t[:, :, :],
                                     xb_t[:, :, :])
                nc.vector.tensor_add(uvr_t[0:krA, :], xr_t[0:krA, 0, :],
                                     xr_t[0:krA, 1, :])
                nc.vector.tensor_sub(uvr_t[32:32 + krA, :],
                                     xr_t[32:32 + krA, 0, :],
                                     xr_t[32:32 + krA, 1, :])

                # stage A: one 3-matmul chain per (rt, parity), moving 1024;
                # the 31-row tail packs yE at partitions 0:31 and yO at
                # 32:63 of ONE tile so a single base-0 cast evacuates both
                y16E = ypool.tile([P, kfB, D], f16, tag="y16E")
                y16O = ypool.tile([P, kfB, D], f16, tag="y16O")
                yr16 = ypool.tile([64, D], f16, tag="yr16")
                return u_t, v_t, uvr_t, y16E, y16O, yr16

            def stage_a_rt(rt, ta):
                u_t, v_t, uvr_t, y16E, y16O, yr16 = ta
                if True:
                    r0 = rt * P
                    rows = min(P, F - r0)
                    tail = rt >= kfB
                    pe = psum_pool.tile([P, D], f32, tag="ps", name="pe")
                    po = pe if tail else psum_pool.tile([P, D], f32,
                                                        tag="ps", name="po")
                    oP = 32 if tail else 0
                    for ng in range(2):
                        n0 = ng * 512
                        n1 = n0 + 512
                        for kt in range(kfA):
                            nc.tensor.matmul(pe[0:rows, n0:n1],
                                             waE_t[:, kt, r0:r0 + rows],
                                             u_t[:, kt, n0:n1],
                                             start=(kt == 0), stop=False)
                            nc.tensor.matmul(po[oP:oP + rows, n0:n1],
                                             waO_t[:, kt, r0:r0 + rows],
                                             v_t[:, kt, n0:n1],
                                             start=(kt == 0), stop=False)
                        nc.tensor.matmul(pe[0:rows, n0:n1],
                                         wrA_t[0:krA, r0:r0 + rows],
                                         uvr_t[0:krA, n0:n1],
                                         start=False, stop=True)
                        nc.tensor.matmul(po[oP:oP + rows, n0:n1],
                                         wrA_t[32:32 + krA, r0:r0 + rows],
                                         uvr_t[32:32 + krA, n0:n1],
                                         start=False, stop=True)
                    # single fp16 cast per tile (ACT) = trunc staging AND
                    # stage-B matmul input
                    if not tail:
                        nc.scalar.copy(y16E[0:rows, rt, :], pe[0:rows, :])
                        nc.scalar.copy(y16O[0:rows, rt, :], po[0:rows, :])
                    else:
                        nc.scalar.copy(yr16[0:32 + rows, :],
                                       pe[0:32 + rows, :])

            def stage_a_out(b, ta):
                y16E, y16O, yr16 = ta[3], ta[4], ta[5]
                # trunc out, fp16 (sync ring): row = 2*(kt*128+p) + parity
                nc.scalar.dma_start(
                    out=tr_d[b, 0:2 * kfB * P:2, :].rearrange(
                        "(kt p) d -> p kt d", p=P),
                    in_=y16E[:, 0:kfB, :])
                nc.scalar.dma_start(
                    out=tr_d[b, 1:2 * kfB * P:2, :].rearrange(
                        "(kt p) d -> p kt d", p=P),
                    in_=y16O[:, 0:kfB, :])
                nc.gpsimd.dma_start(out=tr_d[b, 4 * P:L:2, :],
                                    in_=yr16[0:krB, :])
                nc.gpsimd.dma_start(out=tr_d[b, 4 * P + 1:L:2, :],
                                    in_=yr16[32:32 + krB, :])

            def stage_b_pre():
                a16 = abpool.tile([P, RT, D], f16, tag="a16")
                b16 = abpool.tile([P, RT, D], f16, tag="b16")
                rf_sb = rpool.tile([P, RT, D], f16, tag="rf")
                rb_sb = rpool.tile([P, RT, D], f16, tag="rb")
                return a16, b16, rf_sb, rb_sb

            def stage_b_rt(rt, ta, tb):
                y16E, y16O, yr16 = ta[3], ta[4], ta[5]
                a16, b16, rf_sb, rb_sb = tb
                if True:
                    r0 = rt * P
                    rows = min(P, F - r0)
                    pa = psum_pool.tile([P, D], f32, tag="ps", name="pa")
                    pb = psum_pool.tile([P, D], f32, tag="ps", name="pb")
                    for ng in range(2):
                        n0 = ng * 512
                        n1 = n0 + 512
                        for kt in range(kfB):
                            nc.tensor.matmul(pa[0:rows, n0:n1],
                                             wbE_t[:, kt, r0:r0 + rows],
                                             y16E[:, kt, n0:n1],
                                             start=(kt == 0), stop=False)
                            nc.tensor.matmul(pb[0:rows, n0:n1],
                                             wbO_t[:, kt, r0:r0 + rows],
                                             y16O[:, kt, n0:n1],
                                             start=(kt == 0), stop=False)
                        nc.tensor.matmul(pa[0:rows, n0:n1],
                                         wrB_t[0:krB, r0:r0 + rows],
                                         yr16[0:krB, n0:n1],
                                         start=False, stop=True)
                        nc.tensor.matmul(pb[0:rows, n0:n1],
                                         wrB_t[32:32 + krB, r0:r0 + rows],
                                         yr16[32:32 + krB, n0:n1],
                                         start=False, stop=True)
                    # PSUM evacuation split across both engines so neither
                    # queue gates the tile rotation; combines at fp16 rate
                    nc.scalar.copy(a16[0:rows, rt, :], pa[0:rows, :])
                    nc.vector.tensor_copy(b16[0:rows, rt, :], pb[0:rows, :])
                    nc.vector.tensor_add(rf_sb[0:rows, rt, :],
                                         a16[0:rows, rt, :],
                                         b16[0:rows, rt, :])
                    nc.vector.tensor_sub(rb_sb[0:rows, rt, :],
                                         a16[0:rows, rt, :],
                                         b16[0:rows, rt, :])

            def stage_b_out(b, tb):
                rf_sb, rb_sb = tb[2], tb[3]
                # recon out (gpsimd SWDGE — the known-safe fp16 path)
                nc.gpsimd.dma_start(
                    out=rf_d[b, 0:kfB * P, :].rearrange(
                        "(kt p) d -> p kt d", p=P),
                    in_=rf_sb[:, 0:kfB, :])
                nc.gpsimd.dma_start(out=rf_d[b, kfB * P:F, :],
                                    in_=rf_sb[0:krB, kfB, :])
                nc.gpsimd.dma_start(
                    out=rb_d[b, 0:kfB * P, :].rearrange(
                        "(kt p) d -> p kt d", p=P),
                    in_=rb_sb[:, 0:kfB, :])
                nc.gpsimd.dma_start(out=rb_d[b, kfB * P:F, :],
                                    in_=rb_sb[0:krB, kfB, :])

            # stage B of batch b-1 runs after stage A of batch b so the
            # PE has work while the fp16 casts of batch b drain
            prev = None
            for b in range(Bc):
                ta = stage_a_pre(b)
                for rt in range(RT):
                    stage_a_rt(rt, ta)
                stage_a_out(b, ta)
                if prev is not None:
                    tb = stage_b_pre()
                    for rt in range(RT):
                        stage_b_rt(rt, prev, tb)
                    stage_b_out(b - 1, tb)
                prev = ta
            tb = stage_b_pre()
            for rt in range(RT):
                stage_b_rt(rt, prev, tb)
            stage_b_out(Bc - 1, tb)

    nc.compile()
    return nc


def _numpy_fallback(x):
    """Reference math on host — only for unexpected shapes/degenerate L."""
    B, S, D = x.shape
    C = _dct_matrix64(S).astype(np.float32)
    x_dct = np.tensordot(C, x, axes=([1], [1])).transpose(1, 0, 2)  # (B,S,D)
    m = np.abs(x_dct.mean(axis=0).mean(axis=1))
    thr = np.quantile(m, 0.7)
    idx = np.nonzero(m > thr)[0]
    last_index = int(idx[-1]) if idx.size > 0 else -1
    trunc = x_dct[:, :last_index, :]
    L = trunc.shape[1]
    Cl = _dct_matrix64(L).astype(np.float32)
    recon = np.tensordot(Cl.T, trunc, axes=([1], [1])).transpose(1, 0, 2)
    return recon.astype(np.float16), np.ascontiguousarray(trunc)


def kernel(x, _trace=False):
    global LAST_RESULTS
    x = np.ascontiguousarray(np.asarray(x), dtype=np.float32)
    if x.shape != (_B, _S, _D):
        return _numpy_fallback(x)

    L = _resolve_L(x)
    # butterfly path needs even L with L/2 in (2*P, 3*P)
    if L % 2 or not (2 * _P < L // 2 < 3 * _P) or L >= _S:
        return _numpy_fallback(x)

    Bc = _B // _NCORES
    key = (Bc, _S, _D, L)
    if key not in _CACHE:
        _CACHE[key] = _build_program(Bc, _S, _D, L)
    nc = _CACHE[key]

    _ensure_paths()
    if not _trace:
        os.environ["BASS_NEVER_TRACE"] = "1"
    from concourse.bass_utils import run_bass_kernel_spmd

    W = _build_weights(_S, L)
    H, F = _S // 2, L // 2
    x16 = x.astype(np.float16)
    xf = np.ascontiguousarray(x16[:, :H, :])
    xb = np.ascontiguousarray(x16[:, _S - 1:H - 1:-1, :])
    in_maps = []
    for i in range(_NCORES):
        m = {"xf": xf[i * Bc:(i + 1) * Bc], "xb": xb[i * Bc:(i + 1) * Bc]}
        m.update(W)
        in_maps.append(m)
    res = run_bass_kernel_spmd(nc, in_maps, list(range(_NCORES)), trace=_trace)
    LAST_RESULTS = res

    trunc = np.empty((_B, L, _D), dtype=np.float32)
    recon = np.empty((_B, L, _D), dtype=np.float16)
    for i in range(_NCORES):
        sl = slice(i * Bc, (i + 1) * Bc)
        trunc[sl] = res.results[i]["trunc16"]
        recon[sl, :F] = res.results[i]["rfront"]
        recon[sl, F:] = res.results[i]["rback"][:, ::-1]
    return recon, trunc
